# revision 41
# baseline (speedup 1.0000x reference)
"""MoE feed-forward block (shared expert + top-2-of-8 routed experts) on 8
Trainium2 NeuronCores.

Sharding: expert-parallel with host-side token dispatch (the shard step).
The host computes the top-2 routing and gathers, for core c, the tokens that
routed to expert c (padded to a uniform capacity `cap`) plus a 512-token slice
of the shared expert's work. Each core runs the SAME program (SPMD): a plain
FFN  y = silu(x @ w1.T) @ w2.T  over its token buffer, where the first `cap`
tokens use expert c's weights and the last 512 use the shared-expert weights.
The host applies the per-token gate coefficients while scatter-adding the
per-core outputs into the full result (the unshard step). This does 1/3 of
the dense-MoE FLOPs (3 effective experts per token instead of 9).

Matmuls run in bf16 with fp32 accumulation in PSUM.

Device layout (everything [feature, token]-major, 128-partition stripped):
  mm1: h.T[H,t]  = w1T[D,H].T @ x.T[D,t]     (lhsT=w1T stationary)
  mm2: y.T[D,t]  = w2T[H,D].T @ sh.T[H,t]    (lhsT=w2T stationary)
Each stationary load is shared by the matmuls of all token sub-chunks of the
segment (LDWEIGHTS amortized), and mm2's moving operand is the silu output so
no transposes are needed anywhere. The shared segment runs first (its first
matmul needs only ~0.5MB of DMA); the expert loads are staggered through the
shared phase. All weight DMAs are few, large and CONTIGUOUS (w1 is repacked
group-major on the host): HWDGE descriptor generation (DIRECT2D) serializes
on the Sync sequencer at ~0.6us per contiguous dma_start but ~2.5us per
strided one, which would otherwise gate the w1 stream.
"""

import os

import ml_dtypes
import numpy as np

import concourse.bass as bass
import concourse.mybir as mybir
import concourse.tile as tile
from concourse import bacc
from concourse.bass import ds, ts
from concourse.bass_utils import run_bass_kernel_spmd

BF16 = ml_dtypes.bfloat16

D_MODEL = 1024
HIDDEN = 4096
N_EXP = 8
N_CORES = 8
T = 4096                      # 2 * 2048 tokens
SH = T // N_CORES             # shared-expert tokens per core
P = 128
NG = HIDDEN // 512            # w1 ht-groups (4 h-tiles = 512 cols each)

LAST_EXEC_NS = None
LAST_RESULT = None


def _split(n):
    """Split n tokens into sub-chunks (<=512, multiples of 32, prefer >=2
    pieces so consecutive matmuls share each LDWEIGHTS)."""
    if n <= 128:
        return [n]
    if n <= 512:
        a = (n // 2 + P - 1) // P * P
        return [a, n - a]
    out = []
    while n > 512:
        out.append(512)
        n -= 512
    if n:
        out.append(n)
    return out


def _build_nc(M, cap):
    fp32 = mybir.dt.float32
    bf16 = mybir.dt.bfloat16
    AF = mybir.ActivationFunctionType

    nc = bacc.Bacc()
    # x in contiguous pieces: shared-segment tokens sub-chunk-block-major
    # (so the first matmul's 0.5MB block lands alone, before the second),
    # expert tokens as one piece (prefetched under the shared phase)
    nsh = len(_split(SH))
    xsh = nc.declare_dram_parameter("xsh", [P, nsh, 8, SH // nsh], bf16,
                                    isOutput=False)
    xe = nc.declare_dram_parameter("xe", [P, 8, cap], bf16, isOutput=False)
    # w1 group-major: [P, group, k, 512] so one group's DMA is contiguous
    w1e = nc.declare_dram_parameter("w1e", [P, NG, 8, 512], bf16, isOutput=False)
    w2e = nc.declare_dram_parameter("w2e", [P, 32, D_MODEL], bf16, isOutput=False)
    w1s = nc.declare_dram_parameter("w1s", [P, NG, 8, 512], bf16, isOutput=False)
    w2s = nc.declare_dram_parameter("w2s", [P, 32, D_MODEL], bf16, isOutput=False)
    outT = nc.declare_dram_parameter("outT", [P, 8, M], bf16, isOutput=True)

    # segments: shared first (fast start; expert loads prefetch under it),
    # then the expert tokens, split if cap is large (PSUM budget allows at
    # most 3 sub-chunks per segment; SBUF budget caps segment length, and
    # multi-segment runs use a smaller cap so two x tiles fit double-buffered)
    maxseg = 1280 if cap <= 1280 else 1024
    segs = [(cap, _split(SH), w1s, w2s)]
    off = 0
    while off < cap:
        take = min(maxseg, cap - off)
        segs.append((off, _split(take), w1e, w2e))
        off += take

    emax = max(sum(s[1]) for s in segs[1:])   # longest expert segment
    with tile.TileContext(nc) as tc:
        with (
            tc.tile_pool(name="xs", bufs=1) as xpool,
            tc.tile_pool(name="xep", bufs=1 if len(segs) == 2 else 2) as xepool,
            tc.tile_pool(name="w1p", bufs=4) as w1pool,
            tc.tile_pool(name="w2p", bufs=1) as w2pool,
            tc.tile_pool(name="shp", bufs=1) as shpool,
            tc.tile_pool(name="outp", bufs=2) as opool,
            tc.tile_pool(name="ps", bufs=2, space="PSUM") as pspool,
        ):
            # PE warmup: dummy matmuls run during the initial DMA wait so
            # the HAM clock gate reaches 2.4GHz before the real compute
            # (otherwise the first ~3.4us of matmuls run at 1.2GHz).
            warm = xpool.tile([P, 64], bf16, tag="warm", name="warm")
            pwarm = pspool.tile([P, 64], fp32, tag="pwarm", name="pwarm")
            nc.vector.memset(warm[:], 0)
            for _ in range(100):
                nc.tensor.matmul(pwarm[:64, :], warm[:, :64], warm[:, :64],
                                 start=True, stop=True)

            xsh_sb = xpool.tile([P, nsh, 8, SH // nsh], bf16, tag="xsh",
                                name="xsh")
            # startup DMAs in exact need-order: x block 0 + k=0 weights gate
            # the first matmul, x block 1 is read ~0.2us later (j=1), the
            # remaining k rows of the weight group only from k=1 onward
            w1tile0 = w1pool.tile([P, 8, 512], bf16, tag="w1", name="w1")
            nc.sync.dma_start(xsh_sb[:, 0], xsh[:, 0])
            nc.sync.dma_start(w1tile0[:, 0, :], segs[0][2][:, 0, 0, :])
            for b in range(1, nsh):
                nc.sync.dma_start(xsh_sb[:, b], xsh[:, b])
            nc.sync.dma_start(w1tile0[:, 1:, :], segs[0][2][:, 0, 1:, :])

            w1pre = w1tile0  # current segment's first w1 group, preloaded
            xpre = None      # next expert segment's x tile, prefetched
            seg_max = max(sum(s[1]) for s in segs)
            for si, (base, sizes, w1src, w2src) in enumerate(segs):
                offs = np.cumsum([0] + sizes[:-1]).tolist()
                xe_sb, xpre = xpre, None

                # ---- mm1 + silu: sh.T[H, seg] ----
                shT = shpool.tile([P, HIDDEN // P, seg_max], bf16, tag="shT")
                for ht in range(HIDDEN // P):
                    if ht % 4 == 0:
                        if ht == 0 and w1pre is not None:
                            w1tile = w1pre    # prefetched
                            w1pre = None
                        else:
                            w1tile = w1pool.tile([P, 8, 512], bf16, tag="w1")
                            nc.sync.dma_start(w1tile[:],
                                              w1src[:, ht // 4, :, :])
                    phs = [pspool.tile([P, 512], fp32, tag=f"ph{j}",
                                       name=f"ph{j}")
                           for j in range(len(sizes))]
                    for k in range(8):
                        for j, (off, sz) in enumerate(zip(offs, sizes)):
                            xap = (xsh_sb[:, j, k, :] if si == 0 else
                                   xe_sb[:, k, ds(off, sz)])
                            nc.tensor.matmul(phs[j][:, :sz],
                                             w1tile[:, k, ts(ht % 4, P)],
                                             xap,
                                             start=(k == 0), stop=(k == 7))
                        if si == 0 and ht == 0 and k == 0:
                            # keep the PE busy through the startup DMA wait
                            # (an idle window here re-throttles the clock
                            # gate back to 1.2GHz for the next ~3.4us)
                            for _ in range(30):
                                nc.tensor.matmul(pwarm[:64, :], warm[:, :64],
                                                 warm[:, :64],
                                                 start=True, stop=True)
                    for j, (off, sz) in enumerate(zip(offs, sizes)):
                        nc.scalar.activation(shT[:, ht, ds(off, sz)],
                                             phs[j][:, :sz], AF.Silu)

                # w2 in two D-halves: halves the reload WAR window between
                # segments. The hi half and the next segment's x / first w1
                # group are issued from inside the mm2 loop so each phase's
                # HBM traffic is staggered instead of saturating the link.
                w2h = []
                for h in range(2):
                    w2t = w2pool.tile([P, 32, 512], bf16, tag=f"w2_{h}",
                                      name=f"w2_{h}")
                    w2h.append(w2t)
                nc.sync.dma_start(w2h[0][:], w2src[:, :, ds(0, 512)])
                nxt = segs[si + 1] if si + 1 < len(segs) else None

                # ---- mm2 (transposed): y.T[D, seg] = w2T.T @ sh.T ----
                for dt in range(D_MODEL // P):
                    w2t = w2h[dt // 4]
                    phs = [pspool.tile([P, 512], fp32, tag=f"ph{j}",
                                       name=f"ph{j}")
                           for j in range(len(sizes))]
                    for k in range(HIDDEN // P):
                        for j, (off, sz) in enumerate(zip(offs, sizes)):
                            nc.tensor.matmul(phs[j][:, :sz],
                                             w2t[:, k, ts(dt % 4, P)],
                                             shT[:, k, ds(off, sz)],
                                             start=(k == 0),
                                             stop=(k == HIDDEN // P - 1))
                    # PSUM -> SBUF copies alternating between DVE and ACT.
                    # Last segment: per-sub-chunk output DMAs so the final
                    # transfer (smallest sub-chunk) trails minimally.
                    ysb = opool.tile([P, seg_max], bf16, tag="ysb")
                    for j, (off, sz) in enumerate(zip(offs, sizes)):
                        if j % 2 == 0:
                            nc.vector.tensor_copy(ysb[:, ds(off, sz)],
                                                  phs[j][:, :sz])
                        else:
                            nc.scalar.activation(ysb[:, ds(off, sz)],
                                                 phs[j][:, :sz], AF.Copy)
                        if si == len(segs) - 1:
                            nc.sync.dma_start(outT[:, dt, ds(base + off, sz)],
                                              ysb[:, ds(off, sz)])
                    if si != len(segs) - 1:
                        seg_sz = sum(sizes)
                        nc.sync.dma_start(outT[:, dt, ds(base, seg_sz)],
                                          ysb[:, :seg_sz])
                    # staggered prefetch
                    if dt == 0:
                        nc.sync.dma_start(w2h[1][:],
                                          w2src[:, :, ds(512, 512)])
                    elif dt == 1 and nxt is not None:
                        nlen = sum(nxt[1])
                        xpre = xepool.tile([P, 8, emax], bf16, tag="xe",
                                           name="xe")
                        nc.sync.dma_start(xpre[:, :, :nlen],
                                          xe[:, :, ds(nxt[0], nlen)])
                    elif dt == 2 and nxt is not None:
                        w1pre = w1pool.tile([P, 8, 512], bf16, tag="w1",
                                            name="w1pre")
                        nc.sync.dma_start(w1pre[:], nxt[2][:, 0, :, :])
    nc.compile()
    return nc


def _strip(a, dtype):
    # [K, F] -> [128, K//128, F] partition-major layout
    k, f = a.shape
    return np.ascontiguousarray(
        a.reshape(k // P, P, f).transpose(1, 0, 2)).astype(dtype)


def _w1_prep(w1):
    # torch-layout w1 [H, D] -> [128, NG, 8, 512] (partition, ht-group, k,
    # cols) so each ht-group's DMA is one contiguous transfer
    s = _strip(np.ascontiguousarray(w1.T), BF16)          # [128, 8, H]
    return np.ascontiguousarray(
        s.reshape(P, 8, NG, 512).transpose(0, 2, 1, 3))   # [128, NG, 8, 512]


def kernel(x, shared_w1, shared_w2, experts_w1, experts_w2, gate_w):
    global LAST_EXEC_NS, LAST_RESULT
    x = np.asarray(x, dtype=np.float32).reshape(T, D_MODEL)
    shared_w1 = np.asarray(shared_w1, dtype=np.float32)
    shared_w2 = np.asarray(shared_w2, dtype=np.float32)
    experts_w1 = np.asarray(experts_w1, dtype=np.float32)
    experts_w2 = np.asarray(experts_w2, dtype=np.float32)
    gate_w = np.asarray(gate_w, dtype=np.float32)

    # ---- host-side top-2 routing (the dispatch/shard step) ----
    z = x @ gate_w.T                                    # [T, E] fp32
    ar = np.arange(T)
    i1 = np.argmax(z, axis=1)
    zm = z.copy()
    zm[ar, i1] = -np.inf
    i2 = np.argmax(zm, axis=1)
    z1 = z[ar, i1].astype(np.float64)
    z2 = z[ar, i2].astype(np.float64)
    e2 = np.exp(z2 - z1)
    g1 = (1.0 / (1.0 + e2)).astype(np.float32)
    g2 = (e2 / (1.0 + e2)).astype(np.float32)

    idx_lists, gv_lists = [], []
    for e in range(N_EXP):
        m1 = i1 == e
        m2 = i2 == e
        idx = np.nonzero(m1 | m2)[0]
        gv = np.where(m1, g1, g2)[idx]
        idx_lists.append(idx)
        gv_lists.append(gv)

    n_max = max(len(ix) for ix in idx_lists)
    cap = max(P, ((n_max + 7) // 8) * 8)
    cap = max(cap, int(os.environ.get("BASS_MOE_MIN_CAP", "0")))  # test hook
    M = cap + SH

    sw1q = _w1_prep(shared_w1)
    sw2t = _strip(np.ascontiguousarray(shared_w2.T), BF16)   # [128, 32, D]

    in_maps = []
    for c in range(N_CORES):
        idx = idx_lists[c]
        n = len(idx)
        xc = np.zeros((cap, D_MODEL), dtype=np.float32)
        xc[:n] = x[idx]
        xs = x[c * SH:(c + 1) * SH]
        nsh = len(_split(SH))
        xsq = _strip(np.ascontiguousarray(xs.T), BF16)        # [128, 8, SH]
        xsq = np.ascontiguousarray(
            xsq.reshape(P, 8, nsh, SH // nsh).transpose(0, 2, 1, 3))
        in_maps.append({
            "xe": _strip(np.ascontiguousarray(xc.T), BF16),   # [128, 8, cap]
            "xsh": xsq,                             # [128, nsh, 8, SH//nsh]
            "w1e": _w1_prep(experts_w1[c]),
            "w2e": _strip(np.ascontiguousarray(experts_w2[c].T), BF16),
            "w1s": sw1q, "w2s": sw2t,
        })

    nc = _build_nc(M, cap)
    res = run_bass_kernel_spmd(nc, in_maps, list(range(N_CORES)))
    LAST_EXEC_NS = res.exec_time_ns
    LAST_RESULT = res

    out = np.zeros((T, D_MODEL), dtype=np.float32)
    ys = []
    for c in range(N_CORES):
        yT = np.asarray(res.results[c]["outT"], dtype=np.float32)
        y = yT.transpose(1, 0, 2).reshape(D_MODEL, M).T    # [M, D]
        ys.append(y)
        out[c * SH:(c + 1) * SH] = y[cap:]
    for c in range(N_CORES):
        idx, gv = idx_lists[c], gv_lists[c]
        out[idx] += gv[:, None] * ys[c][:len(idx)]
    return out.reshape(2, 2048, D_MODEL).astype(np.float32)


# revision 42
# speedup vs baseline: 1.0111x; 1.0111x over previous
"""MoE feed-forward block (shared expert + top-2-of-8 routed experts) on 8
Trainium2 NeuronCores.

Sharding: expert-parallel with host-side token dispatch (the shard step).
The host computes the top-2 routing and gathers, for core c, the tokens that
routed to expert c (padded to a uniform capacity `cap`) plus a 512-token slice
of the shared expert's work. Each core runs the SAME program (SPMD): a plain
FFN  y = silu(x @ w1.T) @ w2.T  over its token buffer, where the first `cap`
tokens use expert c's weights and the last 512 use the shared-expert weights.
The host applies the per-token gate coefficients while scatter-adding the
per-core outputs into the full result (the unshard step). This does 1/3 of
the dense-MoE FLOPs (3 effective experts per token instead of 9).

Matmuls run in bf16 with fp32 accumulation in PSUM.

Device layout (everything [feature, token]-major, 128-partition stripped):
  mm1: h.T[H,t]  = w1T[D,H].T @ x.T[D,t]     (lhsT=w1T stationary)
  mm2: y.T[D,t]  = w2T[H,D].T @ sh.T[H,t]    (lhsT=w2T stationary)
Each stationary load is shared by the matmuls of all token sub-chunks of the
segment (LDWEIGHTS amortized), and mm2's moving operand is the silu output so
no transposes are needed anywhere. The shared segment runs first (its first
matmul needs only ~0.5MB of DMA); the expert loads are staggered through the
shared phase. All weight DMAs are few, large and CONTIGUOUS (w1 is repacked
group-major on the host): HWDGE descriptor generation (DIRECT2D) serializes
on the Sync sequencer at ~0.6us per contiguous dma_start but ~2.5us per
strided one, which would otherwise gate the w1 stream.
"""

import os

import ml_dtypes
import numpy as np

import concourse.bass as bass
import concourse.mybir as mybir
import concourse.tile as tile
from concourse import bacc
from concourse.bass import ds, ts
from concourse.bass_utils import run_bass_kernel_spmd

BF16 = ml_dtypes.bfloat16

D_MODEL = 1024
HIDDEN = 4096
N_EXP = 8
N_CORES = 8
T = 4096                      # 2 * 2048 tokens
SH = T // N_CORES             # shared-expert tokens per core
P = 128
NG = HIDDEN // 512            # w1 ht-groups (4 h-tiles = 512 cols each)

LAST_EXEC_NS = None
LAST_RESULT = None


def _split(n):
    """Split n tokens into sub-chunks (<=512, multiples of 32, prefer >=2
    pieces so consecutive matmuls share each LDWEIGHTS)."""
    if n <= 128:
        return [n]
    if n <= 512:
        a = (n // 2 + P - 1) // P * P
        return [a, n - a]
    out = []
    while n > 512:
        out.append(512)
        n -= 512
    if n:
        out.append(n)
    return out


def _build_nc(M, cap):
    fp32 = mybir.dt.float32
    bf16 = mybir.dt.bfloat16
    AF = mybir.ActivationFunctionType

    nc = bacc.Bacc()
    # x in contiguous pieces: shared-segment tokens sub-chunk-block-major
    # (so the first matmul's 0.5MB block lands alone, before the second),
    # expert tokens as one piece (prefetched under the shared phase)
    nsh = len(_split(SH))
    xsh = nc.declare_dram_parameter("xsh", [P, nsh, 8, SH // nsh], bf16,
                                    isOutput=False)
    xe = nc.declare_dram_parameter("xe", [P, 8, cap], bf16, isOutput=False)
    # w1 group-major: [P, group, k, 512] so one group's DMA is contiguous
    w1e = nc.declare_dram_parameter("w1e", [P, NG, 8, 512], bf16, isOutput=False)
    w2e = nc.declare_dram_parameter("w2e", [P, 32, D_MODEL], bf16, isOutput=False)
    w1s = nc.declare_dram_parameter("w1s", [P, NG, 8, 512], bf16, isOutput=False)
    w2s = nc.declare_dram_parameter("w2s", [P, 32, D_MODEL], bf16, isOutput=False)
    outT = nc.declare_dram_parameter("outT", [P, 8, M], bf16, isOutput=True)

    # segments: shared first (fast start; expert loads prefetch under it),
    # then the expert tokens, split if cap is large (PSUM budget allows at
    # most 3 sub-chunks per segment; SBUF budget caps segment length, and
    # multi-segment runs use a smaller cap so two x tiles fit double-buffered)
    maxseg = 1280 if cap <= 1280 else 1024
    segs = [(cap, _split(SH), w1s, w2s)]
    off = 0
    while off < cap:
        take = min(maxseg, cap - off)
        segs.append((off, _split(take), w1e, w2e))
        off += take

    emax = max(sum(s[1]) for s in segs[1:])   # longest expert segment
    with tile.TileContext(nc) as tc:
        with (
            tc.tile_pool(name="xs", bufs=1) as xpool,
            tc.tile_pool(name="xep", bufs=1 if len(segs) == 2 else 2) as xepool,
            tc.tile_pool(name="w1p", bufs=4) as w1pool,
            tc.tile_pool(name="w2p", bufs=1) as w2pool,
            tc.tile_pool(name="shp", bufs=1) as shpool,
            tc.tile_pool(name="outp", bufs=2) as opool,
            tc.tile_pool(name="ps", bufs=2, space="PSUM") as pspool,
        ):
            # PE warmup: dummy matmuls run during the initial DMA wait so
            # the HAM clock gate reaches 2.4GHz before the real compute
            # (otherwise the first ~3.4us of matmuls run at 1.2GHz).
            warm = xpool.tile([P, 64], bf16, tag="warm", name="warm")
            pwarm = pspool.tile([P, 64], fp32, tag="pwarm", name="pwarm")
            nc.vector.memset(warm[:], 0)
            for _ in range(100):
                nc.tensor.matmul(pwarm[:64, :], warm[:, :64], warm[:, :64],
                                 start=True, stop=True)

            xsh_sb = xpool.tile([P, nsh, 8, SH // nsh], bf16, tag="xsh",
                                name="xsh")
            # the first matmul's weights land first (128KB), then its x
            w1tile0 = w1pool.tile([P, 8, 512], bf16, tag="w1", name="w1")
            nc.sync.dma_start(w1tile0[:, 0, :], segs[0][2][:, 0, 0, :])
            nc.sync.dma_start(xsh_sb[:, 0], xsh[:, 0])
            nc.sync.dma_start(w1tile0[:, 1:, :], segs[0][2][:, 0, 1:, :])
            for b in range(1, nsh):
                nc.sync.dma_start(xsh_sb[:, b], xsh[:, b])

            w1pre = w1tile0  # current segment's first w1 group, preloaded
            xpre = None      # next expert segment's x tile, prefetched
            seg_max = max(sum(s[1]) for s in segs)
            for si, (base, sizes, w1src, w2src) in enumerate(segs):
                offs = np.cumsum([0] + sizes[:-1]).tolist()
                xe_sb, xpre = xpre, None

                # ---- mm1 + silu: sh.T[H, seg] ----
                shT = shpool.tile([P, HIDDEN // P, seg_max], bf16, tag="shT")
                for ht in range(HIDDEN // P):
                    if ht % 4 == 0:
                        if ht == 0 and w1pre is not None:
                            w1tile = w1pre    # prefetched
                            w1pre = None
                        else:
                            w1tile = w1pool.tile([P, 8, 512], bf16, tag="w1")
                            nc.sync.dma_start(w1tile[:],
                                              w1src[:, ht // 4, :, :])
                    phs = [pspool.tile([P, 512], fp32, tag=f"ph{j}",
                                       name=f"ph{j}")
                           for j in range(len(sizes))]
                    for k in range(8):
                        for j, (off, sz) in enumerate(zip(offs, sizes)):
                            xap = (xsh_sb[:, j, k, :] if si == 0 else
                                   xe_sb[:, k, ds(off, sz)])
                            nc.tensor.matmul(phs[j][:, :sz],
                                             w1tile[:, k, ts(ht % 4, P)],
                                             xap,
                                             start=(k == 0), stop=(k == 7))
                        if si == 0 and ht == 0 and k == 0:
                            # keep the PE busy through the startup DMA wait
                            # (an idle window here re-throttles the clock
                            # gate back to 1.2GHz for the next ~3.4us)
                            for _ in range(30):
                                nc.tensor.matmul(pwarm[:64, :], warm[:, :64],
                                                 warm[:, :64],
                                                 start=True, stop=True)
                    for j, (off, sz) in enumerate(zip(offs, sizes)):
                        nc.scalar.activation(shT[:, ht, ds(off, sz)],
                                             phs[j][:, :sz], AF.Silu)

                # w2 in two D-halves: halves the reload WAR window between
                # segments. The hi half and the next segment's x / first w1
                # group are issued from inside the mm2 loop so each phase's
                # HBM traffic is staggered instead of saturating the link.
                w2h = []
                for h in range(2):
                    w2t = w2pool.tile([P, 32, 512], bf16, tag=f"w2_{h}",
                                      name=f"w2_{h}")
                    w2h.append(w2t)
                nc.sync.dma_start(w2h[0][:], w2src[:, :, ds(0, 512)])
                nxt = segs[si + 1] if si + 1 < len(segs) else None

                # ---- mm2 (transposed): y.T[D, seg] = w2T.T @ sh.T ----
                for dt in range(D_MODEL // P):
                    w2t = w2h[dt // 4]
                    phs = [pspool.tile([P, 512], fp32, tag=f"ph{j}",
                                       name=f"ph{j}")
                           for j in range(len(sizes))]
                    for k in range(HIDDEN // P):
                        for j, (off, sz) in enumerate(zip(offs, sizes)):
                            nc.tensor.matmul(phs[j][:, :sz],
                                             w2t[:, k, ts(dt % 4, P)],
                                             shT[:, k, ds(off, sz)],
                                             start=(k == 0),
                                             stop=(k == HIDDEN // P - 1))
                    # PSUM -> SBUF copies alternating between DVE and ACT.
                    # Last segment: per-sub-chunk output DMAs so the final
                    # transfer (smallest sub-chunk) trails minimally.
                    ysb = opool.tile([P, seg_max], bf16, tag="ysb")
                    for j, (off, sz) in enumerate(zip(offs, sizes)):
                        if j % 2 == 0:
                            nc.vector.tensor_copy(ysb[:, ds(off, sz)],
                                                  phs[j][:, :sz])
                        else:
                            nc.scalar.activation(ysb[:, ds(off, sz)],
                                                 phs[j][:, :sz], AF.Copy)
                        if si == len(segs) - 1:
                            nc.sync.dma_start(outT[:, dt, ds(base + off, sz)],
                                              ysb[:, ds(off, sz)])
                    if si != len(segs) - 1:
                        seg_sz = sum(sizes)
                        nc.sync.dma_start(outT[:, dt, ds(base, seg_sz)],
                                          ysb[:, :seg_sz])
                    # staggered prefetch
                    if dt == 0:
                        nc.sync.dma_start(w2h[1][:],
                                          w2src[:, :, ds(512, 512)])
                    elif dt == 1 and nxt is not None:
                        nlen = sum(nxt[1])
                        xpre = xepool.tile([P, 8, emax], bf16, tag="xe",
                                           name="xe")
                        nc.sync.dma_start(xpre[:, :, :nlen],
                                          xe[:, :, ds(nxt[0], nlen)])
                    elif dt == 2 and nxt is not None:
                        w1pre = w1pool.tile([P, 8, 512], bf16, tag="w1",
                                            name="w1pre")
                        nc.sync.dma_start(w1pre[:], nxt[2][:, 0, :, :])
    nc.compile()
    return nc


def _strip(a, dtype):
    # [K, F] -> [128, K//128, F] partition-major layout
    k, f = a.shape
    return np.ascontiguousarray(
        a.reshape(k // P, P, f).transpose(1, 0, 2)).astype(dtype)


def _w1_prep(w1):
    # torch-layout w1 [H, D] -> [128, NG, 8, 512] (partition, ht-group, k,
    # cols) so each ht-group's DMA is one contiguous transfer
    s = _strip(np.ascontiguousarray(w1.T), BF16)          # [128, 8, H]
    return np.ascontiguousarray(
        s.reshape(P, 8, NG, 512).transpose(0, 2, 1, 3))   # [128, NG, 8, 512]


def kernel(x, shared_w1, shared_w2, experts_w1, experts_w2, gate_w):
    global LAST_EXEC_NS, LAST_RESULT
    x = np.asarray(x, dtype=np.float32).reshape(T, D_MODEL)
    shared_w1 = np.asarray(shared_w1, dtype=np.float32)
    shared_w2 = np.asarray(shared_w2, dtype=np.float32)
    experts_w1 = np.asarray(experts_w1, dtype=np.float32)
    experts_w2 = np.asarray(experts_w2, dtype=np.float32)
    gate_w = np.asarray(gate_w, dtype=np.float32)

    # ---- host-side top-2 routing (the dispatch/shard step) ----
    z = x @ gate_w.T                                    # [T, E] fp32
    ar = np.arange(T)
    i1 = np.argmax(z, axis=1)
    zm = z.copy()
    zm[ar, i1] = -np.inf
    i2 = np.argmax(zm, axis=1)
    z1 = z[ar, i1].astype(np.float64)
    z2 = z[ar, i2].astype(np.float64)
    e2 = np.exp(z2 - z1)
    g1 = (1.0 / (1.0 + e2)).astype(np.float32)
    g2 = (e2 / (1.0 + e2)).astype(np.float32)

    idx_lists, gv_lists = [], []
    for e in range(N_EXP):
        m1 = i1 == e
        m2 = i2 == e
        idx = np.nonzero(m1 | m2)[0]
        gv = np.where(m1, g1, g2)[idx]
        idx_lists.append(idx)
        gv_lists.append(gv)

    n_max = max(len(ix) for ix in idx_lists)
    cap = max(P, ((n_max + 7) // 8) * 8)
    cap = max(cap, int(os.environ.get("BASS_MOE_MIN_CAP", "0")))  # test hook
    M = cap + SH

    sw1q = _w1_prep(shared_w1)
    sw2t = _strip(np.ascontiguousarray(shared_w2.T), BF16)   # [128, 32, D]

    in_maps = []
    for c in range(N_CORES):
        idx = idx_lists[c]
        n = len(idx)
        xc = np.zeros((cap, D_MODEL), dtype=np.float32)
        xc[:n] = x[idx]
        xs = x[c * SH:(c + 1) * SH]
        nsh = len(_split(SH))
        xsq = _strip(np.ascontiguousarray(xs.T), BF16)        # [128, 8, SH]
        xsq = np.ascontiguousarray(
            xsq.reshape(P, 8, nsh, SH // nsh).transpose(0, 2, 1, 3))
        in_maps.append({
            "xe": _strip(np.ascontiguousarray(xc.T), BF16),   # [128, 8, cap]
            "xsh": xsq,                             # [128, nsh, 8, SH//nsh]
            "w1e": _w1_prep(experts_w1[c]),
            "w2e": _strip(np.ascontiguousarray(experts_w2[c].T), BF16),
            "w1s": sw1q, "w2s": sw2t,
        })

    nc = _build_nc(M, cap)
    res = run_bass_kernel_spmd(nc, in_maps, list(range(N_CORES)))
    LAST_EXEC_NS = res.exec_time_ns
    LAST_RESULT = res

    out = np.zeros((T, D_MODEL), dtype=np.float32)
    ys = []
    for c in range(N_CORES):
        yT = np.asarray(res.results[c]["outT"], dtype=np.float32)
        y = yT.transpose(1, 0, 2).reshape(D_MODEL, M).T    # [M, D]
        ys.append(y)
        out[c * SH:(c + 1) * SH] = y[cap:]
    for c in range(N_CORES):
        idx, gv = idx_lists[c], gv_lists[c]
        out[idx] += gv[:, None] * ys[c][:len(idx)]
    return out.reshape(2, 2048, D_MODEL).astype(np.float32)
